# revision 18
# baseline (speedup 1.0000x reference)
"""GCNConv (D^-1/2 A D^-1/2 X W + b) on 8 Trainium2 NeuronCores.

Strategy (row-sharded over nodes, per the sharding hint):
  - each core owns a [1024, 8192] row block of the adjacency and the matching
    rows of input_feature; weight/bias replicated.
  - prologue: own support rows S = X @ W (bf16, via TensorE transpose of X).
  - the adjacency block is streamed ONCE from HBM (f32, 1MB chunks); per
    chunk it is cast to bf16 on ACT/DVE with the row-sum fused via accum_out,
    then transposed on TensorE (regular matmul vs identity, 4x128 cols per
    PSUM bank) and copied PSUM->SBUF (split DVE/ACT) into a resident bf16
    A^T.
  - after each 128-row m-tile i: d_i = rsqrt(deg+l); own SV rows
    sv = d_i * S_i are scaled locally and AllGathered per m-tile (8 small
    collectives) -- this removes the separate d AllGather entirely; each
    core then loads the gathered [1024,256] SV slab with one strided DMA.
  - the MAIN matmul is interleaved with the stream: matmul (k-tile t,
    own m-tile i') becomes ready when A^T of m-tile i' is transposed and
    the SV slab for phase(t) has gathered.  Ready matmuls are emitted in
    small quanta between transpose quads so TensorE overlaps the stream
    instead of trailing it (the baseline serialized ~55us of matmul after
    the stream).  PSUM: 4 banks accumulate out[m-tile], 4 banks rotate for
    transposes.
  - epilogue scales rows by d_m, adds broadcast bias, stores f32.
"""
import sys
sys.path.insert(0, "/opt/trn_rl_repo")
from contextlib import ExitStack

import numpy as np

import concourse.bass as bass
import concourse.bacc as bacc
import concourse.tile as tile
import concourse.bass_utils as bass_utils
import concourse.mybir as mybir

N_CORES = 8
N = 8192
DIN = 256
DOUT = 256
P = 128
M_LOC = N // N_CORES          # 1024 rows per core
MT = M_LOC // P               # 8 m-tiles per core
KT = N // P                   # 64 k-tiles global
CHUNK = 2048                  # k-chunk per streaming DMA
NCH = N // CHUNK              # 4 chunks per m-tile
NQ = CHUNK // (4 * P)         # 4 k-quads per chunk
GT = KT // 4                  # 16 k-quad groups (atp tiles per m-tile)
F32 = mybir.dt.float32
BF16 = mybir.dt.bfloat16
RG = [list(range(N_CORES))]
Alu = mybir.AluOpType
ActF = mybir.ActivationFunctionType
AxX = mybir.AxisListType.X
import os as _os
MM_QUANTUM = int(_os.environ.get("KQUANT", "6"))   # matmuls per transpose quad
MM_DELAY = int(_os.environ.get("KDELAY", "0"))     # 1: defer AG-dependent
                                                   # matmuls one extra m-tile
VARIANT = _os.environ.get("KVARIANT", "engine")
KCAST = _os.environ.get("KCAST", "split")   # split: ACT even / DVE odd; act
KCOPY = _os.environ.get("KCOPY", "split3")  # split3 | dve | dvegp | 3way
                              # "engine": f32 chunks + ACT/DVE cast w/ fused
                              # rowsum; "dmacast": whole-m-tile SWDGE DMA with
                              # f32->bf16 cast + DVE rowsum


def _emit_body(nc, tc, pools, consts, rep, stage="full"):
    do_transp = stage in ("transp", "nomm", "full")
    do_coll = stage in ("nomm", "full")
    do_mm = stage == "full"
    (natp, natbp, supp, xtp, atpp, svp, dtp, stagep, tpp, mmp, dram) = pools
    (ident, wb, bias_bc, lv, a, x, w, bias, out) = consts
    R = f"r{rep}_"

    # ---- DRAM bounce buffers for the per-m-tile SV AllGathers ----
    svag_in = [dram.tile([P, DOUT], BF16, tag=f"svin{i}", name=R + f"svin{i}")
               for i in range(MT)]
    svag_out = [dram.tile([N_CORES * P, DOUT], BF16, addr_space="Shared",
                          tag=f"svout{i}", name=R + f"svout{i}")
                for i in range(MT)]

    # ---- prologue: own support rows S = X @ W (bf16) ----
    xt = [xtp.tile([P, M_LOC], BF16, tag="xt", name=R + f"xt{dt}")
          for dt in range(DIN // P)]
    for i in range(MT):
        xb = supp.tile([P, DIN], BF16, tag="xb", name=R + f"xb{i}")
        nc.gpsimd.dma_start(xb[:], x.ap()[i * P:(i + 1) * P, :])
        for dt in range(DIN // P):
            ps = tpp.tile([P, 512], F32, tag="tp", name=R + f"xps{i}_{dt}")
            nc.tensor.matmul(ps[:, 0:P], xb[:, dt * P:(dt + 1) * P], ident[:],
                             start=True, stop=True)
            nc.vector.tensor_copy(xt[dt][:, i * P:(i + 1) * P], ps[:, 0:P])
    sown = []
    for i in range(MT):
        sps_t = tpp.tile([P, 512], F32, tag="tp", name=R + f"sps{i}")
        sps = sps_t[:, 0:DOUT]
        for dt in range(DIN // P):
            nc.tensor.matmul(sps, xt[dt][:, i * P:(i + 1) * P], wb[dt][:],
                             start=(dt == 0), stop=(dt == DIN // P - 1))
        sst = supp.tile([P, DOUT], BF16, tag="sown", name=R + f"sown{i}")
        nc.scalar.copy(sst[:], sps)
        sown.append(sst)

    # ---- resident transposed adjacency (bf16) and gathered SV slabs ----
    atp = {}
    for g in range(GT):
        for i in range(MT):
            atp[(g, i)] = atpp.tile([P, 512], BF16, tag="atp",
                                    name=R + f"atp_{g}_{i}")
    sv = [svp.tile([P, N_CORES * DOUT], BF16, tag="sv", name=R + f"sv{i}")
          for i in range(MT)]

    par = dtp.tile([P, MT * NCH], F32, tag="par", name=R + "par")
    dcols = dtp.tile([P, MT], F32, tag="dcols", name=R + "dcols")

    # ---- main-matmul scheduler state ----
    # one [128,512] psum bank holds two adjacent m-tiles' accumulators
    mmps = [mmp.tile([P, 512], F32, tag="mmps", name=R + f"mmps_{b}")
            for b in range(MT // 2)]
    pending = []          # (t, ip) matmuls whose inputs are (or will be) live
    deferred = []         # AG-dependent matmuls held back one m-tile
    bank_started = [False] * (MT // 2)
    mm_left = [KT] * MT   # matmuls remaining per own m-tile

    def emit_epilogue(ip):
        src = mmps[ip // 2][:, (ip % 2) * DOUT:(ip % 2 + 1) * DOUT]
        st1 = stagep.tile([P, DOUT], F32, tag="stage", name=R + f"st1_{ip}")
        nc.vector.tensor_scalar_mul(st1[:], src, dcols[:, ip:ip + 1])
        st2 = stagep.tile([P, DOUT], F32, tag="stage", name=R + f"st2_{ip}")
        nc.vector.tensor_add(st2[:], st1[:], bias_bc[:])
        nc.sync.dma_start(out.ap()[ip * P:(ip + 1) * P, :], st2[:])

    def emit_mm(t, ip):
        ph, r = t % MT, t // MT
        b = ip // 2
        first = not bank_started[b]
        bank_started[b] = True
        dst = mmps[b][:, (ip % 2) * DOUT:(ip % 2 + 1) * DOUT]
        # start=True clears the WHOLE bank's has_written bits, so only the
        # bank's first matmul (even half) may carry it; the odd half's first
        # matmul overwrites via the cleared bits.
        nc.tensor.matmul(
            dst,
            atp[(t // 4, ip)][:, (t % 4) * P:(t % 4 + 1) * P],
            sv[ph][:, r * DOUT:(r + 1) * DOUT],
            start=first, stop=(mm_left[ip] == 1),
            skip_group_check=True)
        mm_left[ip] -= 1
        if mm_left[ip] == 0:
            emit_epilogue(ip)

    def emit_some(budget):
        while budget > 0 and pending:
            emit_mm(*pending.pop(0))
            budget -= 1

    def emit_copy(g, dst, ps):
        if KCOPY == "dve":
            nc.vector.tensor_copy(dst, ps)
        elif KCOPY == "dvegp":
            if g % 2 == 0:
                nc.vector.tensor_copy(dst, ps)
            else:
                nc.gpsimd.tensor_copy(dst, ps)
        elif KCOPY == "3way":
            e = g % 3
            if e == 0:
                nc.vector.tensor_copy(dst, ps)
            elif e == 1:
                nc.scalar.copy(dst, ps)
            else:
                nc.gpsimd.tensor_copy(dst, ps)
        else:  # split3
            if g % 3 == 2:
                nc.scalar.copy(dst, ps)
            else:
                nc.vector.tensor_copy(dst, ps)

    # ---- stream the adjacency block ----
    for i in range(MT):
        deg = dtp.tile([P, 1], F32, tag="deg", bufs=2, name=R + f"deg{i}")
        if VARIANT == "dmacast":
            DC = 4096                 # cast-DMA chunk (2MB f32 read)
            NDC = N // DC
            NQD = DC // (4 * P)
            for j in range(NDC):
                nat = natbp.tile([P, DC], BF16, tag="natb",
                                 name=R + f"nat{i}_{j}")
                nc.gpsimd.dma_start(
                    nat[:], a.ap()[i * P:(i + 1) * P, j * DC:(j + 1) * DC])
                c = i * NDC + j
                nc.vector.tensor_reduce(par[:, c:c + 1], nat[:], axis=AxX,
                                        op=Alu.add)
                for q in (range(NQD) if do_transp else ()):
                    g = j * NQD + q
                    ps = tpp.tile([P, 512], F32, tag="tp",
                                  name=R + f"tps{i}_{g}")
                    for u in range(4):
                        s = q * 4 + u
                        nc.tensor.matmul(ps[:, u * P:(u + 1) * P],
                                         nat[:, s * P:(s + 1) * P], ident[:],
                                         start=True, stop=True)
                    if g % 3 == 2:
                        nc.scalar.copy(atp[(g, i)][:], ps[:])
                    else:
                        nc.vector.tensor_copy(atp[(g, i)][:], ps[:])
                    if do_mm:
                        emit_some(MM_QUANTUM)
            nc.vector.tensor_reduce(deg[:], par[:, i * NDC:(i + 1) * NDC],
                                    axis=AxX, op=Alu.add)
        else:
            for j in range(NCH):
                natf = natp.tile([P, CHUNK], F32, tag="nat",
                                 name=R + f"natf{i}_{j}")
                nc.sync.dma_start(
                    natf[:],
                    a.ap()[i * P:(i + 1) * P, j * CHUNK:(j + 1) * CHUNK])
                nat = natbp.tile([P, CHUNK], BF16, tag="natb",
                                 name=R + f"nat{i}_{j}")
                c = i * NCH + j
                if KCAST == "act" or j % 2 == 0:
                    nc.scalar.activation(nat[:], natf[:], ActF.Copy,
                                         accum_out=par[:, c:c + 1])
                else:
                    nc.vector.tensor_scalar(nat[:], natf[:], 1.0, None,
                                            op0=Alu.mult, op1=Alu.add,
                                            accum_out=par[:, c:c + 1])
                for q in (range(NQ) if do_transp else ()):
                    g = j * NQ + q
                    ps = tpp.tile([P, 512], F32, tag="tp",
                                  name=R + f"tps{i}_{g}")
                    for u in range(4):
                        s = q * 4 + u
                        nc.tensor.matmul(ps[:, u * P:(u + 1) * P],
                                         nat[:, s * P:(s + 1) * P], ident[:],
                                         start=True, stop=True)
                    emit_copy(g, atp[(g, i)][:], ps[:])
                    if do_mm:
                        emit_some(MM_QUANTUM)
        # ---- end of m-tile i: degree -> d_i ----
        if VARIANT != "dmacast":
            nc.vector.tensor_reduce(deg[:], par[:, i * NCH:(i + 1) * NCH],
                                    axis=AxX, op=Alu.add)
        deg2 = dtp.tile([P, 1], F32, tag="deg2", bufs=2, name=R + f"deg2{i}")
        nc.vector.tensor_scalar_add(deg2[:], deg[:], lv[:])
        s0 = dtp.tile([P, 1], F32, tag="s0", bufs=2, name=R + f"s0{i}")
        nc.scalar.sqrt(s0[:], deg2[:])
        r0 = dtp.tile([P, 1], F32, tag="r0", bufs=2, name=R + f"r0{i}")
        nc.vector.reciprocal(r0[:], s0[:])
        # one Newton step: d = r0 * (1.5 - 0.5 * deg2 * r0^2)
        t1 = dtp.tile([P, 1], F32, tag="t1", bufs=2, name=R + f"t1{i}")
        nc.vector.tensor_mul(t1[:], r0[:], r0[:])
        t2 = dtp.tile([P, 1], F32, tag="t2", bufs=2, name=R + f"t2{i}")
        nc.vector.tensor_mul(t2[:], t1[:], deg2[:])
        t3 = dtp.tile([P, 1], F32, tag="t3", bufs=2, name=R + f"t3{i}")
        nc.vector.tensor_scalar(t3[:], t2[:], -0.5, 1.5, op0=Alu.mult,
                                op1=Alu.add)
        nc.vector.tensor_mul(dcols[:, i:i + 1], r0[:], t3[:])
        if do_coll:
            # own SV rows, scaled locally -- no d AllGather needed
            svo = supp.tile([P, DOUT], BF16, tag="svo", bufs=2,
                            name=R + f"svo{i}")
            nc.vector.tensor_scalar_mul(svo[:], sown[i][:], dcols[:, i:i + 1])
            nc.sync.dma_start(svag_in[i][:], svo[:])
            nc.gpsimd.collective_compute(
                "AllGather", Alu.bypass, replica_groups=RG,
                ins=[svag_in[i].opt()], outs=[svag_out[i].opt()])
            nc.gpsimd.dma_start(
                sv[i][:],
                svag_out[i][:].rearrange("(r p) n -> p r n",
                                         r=N_CORES, p=P))
        if do_mm:
            # newly-ready matmuls: old phases x new atp first (they need no
            # fresh collective), then the new phase across all ready atp
            if MM_DELAY:
                pending.extend(deferred)
                deferred = []
            for ph in range(i):
                for r in range(N_CORES):
                    pending.append((r * MT + ph, i))
            newph = [(r * MT + i, ip)
                     for r in range(N_CORES) for ip in range(i + 1)]
            if MM_DELAY:
                deferred = newph
            else:
                pending.extend(newph)

    # ---- drain remaining matmuls (epilogues fire inline per m-tile) ----
    if do_mm:
        pending.extend(deferred)
        emit_some(len(pending))
        assert all(v == 0 for v in mm_left)
    else:
        # partial-stage builds still need the output written
        st = stagep.tile([P, DOUT], F32, tag="stage", name=R + "stz")
        nc.vector.memset(st[:], 0.0)
        for i in range(MT):
            nc.sync.dma_start(out.ap()[i * P:(i + 1) * P, :], st[:])


def build(repeat=1, stage="full"):
    nc = bacc.Bacc("TRN2", target_bir_lowering=False, debug=False,
                   num_devices=N_CORES)
    a = nc.dram_tensor("a", [M_LOC, N], F32, kind="ExternalInput")
    x = nc.dram_tensor("x", [M_LOC, DIN], F32, kind="ExternalInput")
    w = nc.dram_tensor("w", [DIN, DOUT], F32, kind="ExternalInput")
    bias = nc.dram_tensor("bias", [DOUT], F32, kind="ExternalInput")
    lvec = nc.dram_tensor("lvec", [P, 1], F32, kind="ExternalInput")
    out = nc.dram_tensor("out", [M_LOC, DOUT], F32, kind="ExternalOutput")

    with tile.TileContext(nc) as tc, ExitStack() as ctx:
        cpool = ctx.enter_context(tc.tile_pool(name="cpool", bufs=1))
        natp = ctx.enter_context(tc.tile_pool(name="natp", bufs=2))
        natbp = ctx.enter_context(tc.tile_pool(name="natbp", bufs=3))
        supp = ctx.enter_context(tc.tile_pool(name="supp", bufs=MT))
        xtp = ctx.enter_context(tc.tile_pool(name="xtp", bufs=2))
        atpp = ctx.enter_context(tc.tile_pool(name="atpp", bufs=GT * MT))
        svp = ctx.enter_context(tc.tile_pool(name="svp", bufs=MT))
        dtp = ctx.enter_context(tc.tile_pool(name="dtp", bufs=1))
        stagep = ctx.enter_context(tc.tile_pool(name="stagep", bufs=2))
        tpp = ctx.enter_context(tc.tile_pool(name="tpp", bufs=4, space="PSUM"))
        mmp = ctx.enter_context(tc.tile_pool(name="mmp", bufs=MT // 2,
                                             space="PSUM"))
        dram = ctx.enter_context(tc.tile_pool(name="dram", bufs=1,
                                              space="DRAM"))

        # ---- constants ----
        ones_bf = cpool.tile([P, P], BF16)
        nc.vector.memset(ones_bf[:], 1.0)
        ident = cpool.tile([P, P], BF16)
        nc.gpsimd.affine_select(
            ident[:], ones_bf[:], pattern=[[1, P]],
            compare_op=Alu.is_equal, fill=0.0, base=0, channel_multiplier=-1)
        wb = []
        for dt in range(DIN // P):
            wt = cpool.tile([P, DOUT], BF16, tag=f"wb{dt}", name=f"wb{dt}")
            nc.gpsimd.dma_start(wt[:], w.ap()[dt * P:(dt + 1) * P, :])
            wb.append(wt)
        lv = cpool.tile([P, 1], F32, tag="lv")
        nc.scalar.dma_start(lv[:], lvec.ap())
        # broadcast bias over partitions with a K=1 matmul
        ones_row = cpool.tile([1, P], F32, tag="ones_row")
        nc.vector.memset(ones_row[:], 1.0)
        bias_row = cpool.tile([1, DOUT], F32, tag="bias_row")
        nc.scalar.dma_start(bias_row[:], bias.ap()[None, :])
        bias_bc = cpool.tile([P, DOUT], F32, tag="bias_bc")
        bps = tpp.tile([P, 512], F32, tag="tp", name="bias_ps")
        nc.tensor.matmul(bps[:, 0:DOUT], ones_row[:], bias_row[:],
                         start=True, stop=True)
        nc.vector.tensor_copy(bias_bc[:], bps[:, 0:DOUT])

        pools = (natp, natbp, supp, xtp, atpp, svp, dtp, stagep, tpp, mmp,
                 dram)
        consts = (ident, wb, bias_bc, lv, a, x, w, bias, out)
        for rep in range(repeat):
            _emit_body(nc, tc, pools, consts, rep, stage=stage)
    nc.compile()
    return nc


def make_in_maps(adjacency, input_feature, weight, bias, l):
    adjacency = np.ascontiguousarray(np.asarray(adjacency, dtype=np.float32))
    input_feature = np.ascontiguousarray(
        np.asarray(input_feature, dtype=np.float32))
    weight = np.ascontiguousarray(np.asarray(weight, dtype=np.float32))
    bias_np = np.ascontiguousarray(np.asarray(bias, dtype=np.float32))
    lval = float(np.asarray(l))
    lv = np.full((P, 1), lval, dtype=np.float32)
    in_maps = []
    for c in range(N_CORES):
        in_maps.append({
            "a": adjacency[c * M_LOC:(c + 1) * M_LOC, :],
            "x": input_feature[c * M_LOC:(c + 1) * M_LOC, :],
            "w": weight,
            "bias": bias_np,
            "lvec": lv,
        })
    return in_maps


_NC_CACHE = None


def kernel(adjacency, input_feature, weight, bias, l):
    global _NC_CACHE
    if _NC_CACHE is None:
        _NC_CACHE = build()
    nc = _NC_CACHE
    in_maps = make_in_maps(adjacency, input_feature, weight, bias, l)
    res = None
    last_err = None
    for attempt in range(3):
        try:
            res = bass_utils.run_bass_kernel_spmd(
                nc, in_maps, core_ids=list(range(N_CORES)))
            break
        except Exception as e:           # transient device wedge: retry
            last_err = e
            import time as _time
            _time.sleep(5.0 * (attempt + 1))
    if res is None:
        raise last_err
    blocks = [res.results[c]["out"] for c in range(N_CORES)]
    return np.ascontiguousarray(np.concatenate(blocks, axis=0),
                                dtype=np.float32)


if __name__ == "__main__":
    rng = np.random.default_rng(0)
    A = rng.random((N, N), dtype=np.float32)
    X = rng.standard_normal((N, DIN)).astype(np.float32)
    W = (rng.standard_normal((DIN, DOUT)) / np.sqrt(DIN)).astype(np.float32)
    B = np.zeros((DOUT,), dtype=np.float32)
    out = kernel(A, X, W, B, 1)
    deg = A.sum(axis=1) + 1.0
    d = np.where(deg > 0, deg ** -0.5, 0.0).astype(np.float32)
    ref = (A * d[:, None] * d[None, :]) @ (X @ W) + B
    err = np.abs(out - ref)
    rel = np.linalg.norm(out - ref) / np.linalg.norm(ref)
    print(f"max abs err {err.max():.3e}  rel l2 {rel:.3e}")
